# revision 1
# baseline (speedup 1.0000x reference)
"""Trainium2 Bass kernel for causal multi-head attention (B=4, T=2048, C=1024, H=16).

Sharding: head-parallel across 8 cores (2 heads per core). Each core computes
its heads' QKV projection, causal attention, and a partial (row-parallel)
output projection; the host sums the 8 partial projections (free vs. HW time).

v3: all matmul operands fp16 (PE streams fp16 ~1.7x faster than f32r on HW).
Attention runs in 512-token q-chunks with BOTH heads' S^T blocks adjacent in
one PSUM tile [128, 2, 512], so each softmax exp is ONE activation
instruction with a 3D AP covering both heads (fewer ScalarE instructions —
the binding engine), and the causal zero-fill / diagonal mask are single
merged DVE ops. PSUM rotates a 3-deep [128,2,512] pool for PE overlap.

Per-core dataflow:
  - x^T fed host-pre-transposed fp16, chunk-major (8KB DMA runs).
  - Q^T, K^T produced as [d2=128, T] per batch (d on partitions); V^T is
    PE-transposed back to V [T-tile, d] blocks (AV lhsT), ones col appended
    so the AV matmul also emits the softmax denominator as row 64 of y^T.
  - S^T[k, q] = K^T(tile) * Q^T per head; the two heads' QK matmuls are
    emitted adjacently on disjoint PE row groups (K=64 at partitions 0-63 /
    64-127) so they run concurrently.
  - Causality: k-tiles above the diagonal never enter the j-loop; exp starts
    at the diagonal column; left-of-diagonal is zero-filled and the diagonal
    128x128 block gets a triangular mask multiply (merged across heads).
  - Normalization: PE-transpose y^T blocks to [q, d], scale rows by the
    reciprocal denominators, PE-transpose back into y2^T [d2=128, T].
  - Output written fp16 in a permuted tile-major layout (8KB DMA runs on the
    gpsimd ring, overlapping the sync-ring input stream); host un-permutes.
"""

import sys
import numpy as np

sys.path.insert(0, "/opt/trn_rl_repo")

B, T, C = 4, 2048, 1024
H = 16
D = C // H            # 64
NCORES = 8
HPC = H // NCORES     # heads per core = 2
D2 = HPC * D          # 128
P = 128
KC = C // P           # 8 contraction tiles for the projections
PC = 512              # qkv production chunk (tokens)
QC = 512              # attention q chunk
NT = T // P           # 16 k-tiles per batch

_CACHE = {}


def build_program():
    import concourse.bacc as bacc
    import concourse.mybir as mybir
    from concourse import tile

    F32R = mybir.dt.float32r
    F32 = mybir.dt.float32
    F16 = mybir.dt.float16
    EXP = mybir.ActivationFunctionType.Exp

    nc = bacc.Bacc(None, target_bir_lowering=False, debug=False)

    # chunk-major so each partition's DMA run is KC*PC*2 = 8KB contiguous
    xT = nc.declare_dram_parameter(
        "xT", [B * T // PC, P, KC, PC], F16, isOutput=False)
    wq = nc.declare_dram_parameter("wq", [P, KC, D2], F16, isOutput=False)
    wk = nc.declare_dram_parameter("wk", [P, KC, D2], F16, isOutput=False)
    wv = nc.declare_dram_parameter("wv", [P, KC, D2], F16, isOutput=False)
    wp = nc.declare_dram_parameter("wp", [P, C], F16, isOutput=False)
    tri2 = nc.declare_dram_parameter("tri2", [P, 2, P], F16, isOutput=False)
    zero2 = nc.declare_dram_parameter("zero2", [P, 2, 384], F16, isOutput=False)
    idin = nc.declare_dram_parameter("idin", [P, P], F16, isOutput=False)
    vconst = nc.declare_dram_parameter("vconst", [P, NT, 2], F16, isOutput=False)
    # permuted output layout: out[p, g, f, :] = row (g*4+f)*128 + p
    # (host un-permutes); fp16 gives 8KB contiguous runs per partition
    out = nc.declare_dram_parameter(
        "out", [P, B * T // (4 * P), 4, C], F16, isOutput=True)

    with tile.TileContext(nc) as tc:
        with (
            tc.tile_pool(name="const", bufs=1) as const,
            tc.tile_pool(name="xtp", bufs=4) as xtp,
            tc.tile_pool(name="qkv", bufs=2) as qkvp,
            tc.tile_pool(name="expp", bufs=6) as expp,
            tc.tile_pool(name="yp", bufs=3) as ypool,
            tc.tile_pool(name="ynp", bufs=4) as ynp,
            tc.tile_pool(name="y2p", bufs=2) as y2p,
            tc.tile_pool(name="outp", bufs=3) as outp,
            tc.tile_pool(name="vsp", bufs=4) as vsp,
            tc.tile_pool(name="recp", bufs=6) as recp,
            tc.tile_pool(name="ps", bufs=3, space="PSUM") as ps,
        ):
            wq_sb = const.tile([P, KC, D2], F16, tag="wq")
            wk_sb = const.tile([P, KC, D2], F16, tag="wk")
            wv_sb = const.tile([P, KC, D2], F16, tag="wv")
            wp_sb = const.tile([P, C], F16, tag="wp")
            tri_sb = const.tile([P, 2, P], F16, tag="tri")
            zero_sb = const.tile([P, 2, 384], F16, tag="zeros")
            ident = const.tile([P, P], F16, tag="ident")
            vc_sb = const.tile([P, NT, 2], F16, tag="vc")
            # critical-path constants first; bulky non-critical ones are
            # deferred until after the first x chunk is in flight
            nc.scalar.dma_start(out=wq_sb[:], in_=wq[:])
            nc.scalar.dma_start(out=wk_sb[:], in_=wk[:])
            nc.scalar.dma_start(out=wv_sb[:], in_=wv[:])
            nc.scalar.dma_start(out=ident[:], in_=idin[:])
            nc.scalar.dma_start(out=vc_sb[:], in_=vconst[:])
            deferred_consts = [(wp_sb, wp), (tri_sb, tri2), (zero_sb, zero2)]

            for b in range(B):
                # ---------------- Phase A: QKV projection for batch b --------
                qt_sb = qkvp.tile([P, T], F16, tag="qt")
                kt_sb = qkvp.tile([P, T], F16, tag="kt")
                # V blocks: [tok-tile p, 2*66] per k-tile:
                #   cols 0:64 head-A dims, 64 ones, 65 zero,
                #   cols 66:130 head-B dims, 130 ones, 131 zero
                v_sb = qkvp.tile([P, NT, 132], F16, tag="v")
                nc.gpsimd.tensor_copy(v_sb[:, :, 64:66], vc_sb[:])
                nc.gpsimd.tensor_copy(v_sb[:, :, 130:132], vc_sb[:])

                for ch in range(T // PC):
                    gch = (b * T) // PC + ch
                    xt = xtp.tile([P, KC, PC], F16, tag="xt")
                    nc.sync.dma_start(out=xt[:], in_=xT[gch])
                    if deferred_consts:
                        dst, src = deferred_consts.pop(0)
                        nc.sync.dma_start(out=dst[:], in_=src[:])
                    for which, w_sb in (("q", wq_sb), ("k", wk_sb), ("v", wv_sb)):
                        pt = ps.tile([P, 2, 512], F32, tag="ps")
                        for kc in range(KC):
                            nc.tensor.matmul(
                                pt[:, 0, :], w_sb[:, kc, :], xt[:, kc, :],
                                start=(kc == 0), stop=(kc == KC - 1),
                            )
                        if which == "q":
                            nc.vector.tensor_copy(
                                qt_sb[:, ch * PC:(ch + 1) * PC], pt[:, 0, :])
                        elif which == "k":
                            nc.vector.tensor_copy(
                                kt_sb[:, ch * PC:(ch + 1) * PC], pt[:, 0, :])
                        else:
                            vts = vsp.tile([P, PC], F16, tag="vts")
                            nc.vector.tensor_copy(vts[:], pt[:, 0, :])
                            # 4 transposes into one psum tile, then 2 batched
                            # strided copies instead of 8 small ones
                            tpsv = ps.tile([P, 16, P], F16, tag="ps",
                                           name="tps")
                            for i in range(PC // P):
                                nc.tensor.transpose(
                                    tpsv[:, i, :], vts[:, i * P:(i + 1) * P],
                                    ident[:])
                            tt0 = ch * (PC // P)
                            nc.vector.tensor_copy(
                                v_sb[:, tt0:tt0 + 4, 0:64],
                                tpsv[:, 0:4, 0:64])
                            nc.vector.tensor_copy(
                                v_sb[:, tt0:tt0 + 4, 66:130],
                                tpsv[:, 0:4, 64:128])

                # ---------------- Phase B: attention for batch b -------------
                y2t_sb = y2p.tile([P, T], F16, tag="y2t")
                for ci in range(T // QC):
                    q0 = ci * QC
                    # AV accumulators, one bank per head (row 64 = denom)
                    yta = ps.tile([66, 512], F32, tag="yta", bufs=1,
                                  name="yta")
                    ytb = ps.tile([66, 512], F32, tag="ytb", bufs=1,
                                  name="ytb")
                    yts2 = (yta, ytb)
                    njt = 4 * (ci + 1)  # k-tiles in the causal span
                    for j in range(njt):
                        st = ps.tile([P, 2, 512], F32, tag="ps", name="st")
                        # two heads on disjoint PE row groups, emitted
                        # adjacently so the K=64 matmuls run concurrently
                        for h in range(HPC):
                            hp0 = h * D
                            nc.tensor.matmul(
                                st[:, h, :],
                                kt_sb[hp0:hp0 + D, j * P:(j + 1) * P],
                                qt_sb[hp0:hp0 + D, q0:q0 + QC],
                                start=True, stop=True,
                            )
                        c0 = max(0, 128 * j - q0)
                        et = expp.tile([P, 2, 512], F16, tag="exp", name="et")
                        # ONE exp instruction covers both heads (3D AP)
                        nc.scalar.activation(
                            et[:, :, c0:512], st[:, :, c0:512], EXP,
                            scale=float(1.0 / np.sqrt(D)))
                        if c0 > 0:
                            nc.vector.tensor_copy(
                                et[:, :, 0:c0], zero_sb[:, :, 0:c0])
                        if 128 * j >= q0:
                            # block contains the diagonal: triangular mask
                            nc.vector.tensor_mul(
                                et[:, :, c0:c0 + 128],
                                et[:, :, c0:c0 + 128],
                                tri_sb[:])
                        for h in range(HPC):
                            nc.tensor.matmul(
                                yts2[h][0:66, :],
                                v_sb[:, j, 66 * h:66 * h + 66],
                                et[:, h, :],
                                start=(j == 0), stop=(j == njt - 1),
                            )
                    # ---- normalize + build y2^T for this q-chunk ----
                    ya_sb = ypool.tile([66, 2, 512], F16, tag="ya")
                    nc.vector.tensor_copy(ya_sb[:, 0, :], yta[:])
                    nc.scalar.copy(ya_sb[:, 1, :], ytb[:])
                    y2ps = ps.tile([P, 2, 1024], F16, tag="ps", name="y2ps")
                    # all 8 [q,d] transposes into ONE psum tile, then ONE bulk
                    # copy + ONE strided reciprocal + fast SBUF fp16 muls
                    # (replaces 128 tiny bubble-dominated recip/mul instrs)
                    tpsa = ps.tile([P, 2, 1024], F16, tag="ps", name="tpsa")
                    for blk in range(4):
                        nc.tensor.transpose(
                            tpsa[:, 0, blk * 132:blk * 132 + 66],
                            ya_sb[0:66, 0, blk * P:(blk + 1) * P],
                            ident[0:66, 0:66])
                        nc.tensor.transpose(
                            tpsa[:, 0, blk * 132 + 66:blk * 132 + 132],
                            ya_sb[0:66, 1, blk * P:(blk + 1) * P],
                            ident[0:66, 0:66])
                    sbt = ynp.tile([P, 528], F16, tag="yn")
                    nc.vector.tensor_copy(sbt[:], tpsa[:, 0, 0:528])
                    rec = recp.tile([P, 8], F32, tag="rec")
                    nc.vector.reciprocal(rec[:], sbt[:, 64:528:66])
                    yn = ynp.tile([P, 4, P], F16, tag="yn2")
                    for blk in range(4):
                        nc.vector.tensor_scalar_mul(
                            yn[:, blk, 0:64],
                            sbt[:, blk * 132:blk * 132 + 64],
                            rec[:, 2 * blk:2 * blk + 1])
                        nc.vector.tensor_scalar_mul(
                            yn[:, blk, 64:128],
                            sbt[:, blk * 132 + 66:blk * 132 + 130],
                            rec[:, 2 * blk + 1:2 * blk + 2])
                    for blk in range(4):
                        nc.tensor.transpose(
                            y2ps[:, 0, blk * P:(blk + 1) * P], yn[:, blk, :],
                            ident[:])
                    nc.vector.tensor_copy(
                        y2t_sb[:, q0:q0 + QC], y2ps[:, 0, 0:QC])

                    # ---- partial out projection for this q-chunk ----
                    osb = outp.tile([P, 4, 2, 512], F16, tag="osb")
                    for f in range(4):
                        ttk = ci * 4 + f
                        pps = ps.tile([P, 2, 512], F32, tag="ps", name="pps")
                        for s in range(2):
                            nc.tensor.matmul(
                                pps[:, s, :],
                                y2t_sb[:, ttk * P:(ttk + 1) * P],
                                wp_sb[:, s * 512:(s + 1) * 512],
                                start=True, stop=True,
                            )
                        if f % 2 == 0:
                            nc.scalar.copy(osb[:, f], pps[:])
                        else:
                            nc.vector.tensor_copy(osb[:, f], pps[:])
                    if b == B - 1 and ci == T // QC - 1:
                        for f in range(4):
                            nc.gpsimd.dma_start(
                                out=out[:, b * 4 + ci, f, :], in_=osb[:, f])
                    else:
                        nc.gpsimd.dma_start(
                            out=out[:, b * 4 + ci, :, :], in_=osb[:])

    nc.compile()
    return nc


def _prepare_inputs(x, w_attn, w_proj):
    xf = np.ascontiguousarray(x.reshape(B * T, C))
    # xT[ch, p, kc, t] = xf[ch*PC + t, kc*128 + p]
    xT = np.ascontiguousarray(
        xf.reshape(B * T // PC, PC, KC, P).transpose(0, 3, 2, 1)).astype(np.float16)

    kk = np.arange(P)[:, None]
    qq = np.arange(P)[None, :]
    tri = (qq >= kk).astype(np.float16)           # [128, 128] causal block
    tri2 = np.ascontiguousarray(
        np.broadcast_to(tri[:, None, :], (P, 2, P)))
    zero2 = np.zeros((P, 2, 384), dtype=np.float16)

    ident = np.eye(P, dtype=np.float16)
    vconst = np.zeros((P, NT, 2), dtype=np.float16)
    vconst[:, :, 0] = 1.0

    in_maps = []
    for c in range(NCORES):
        cols = slice(c * D2, (c + 1) * D2)
        wqa = w_attn[:, cols]
        wka = w_attn[:, C:][:, cols]
        wva = w_attn[:, 2 * C:][:, cols]

        def wt(w):
            return np.ascontiguousarray(
                w.reshape(KC, P, D2).transpose(1, 0, 2)).astype(np.float16)

        wpa = np.ascontiguousarray(w_proj[c * D2:(c + 1) * D2, :]).astype(np.float16)
        in_maps.append({
            "xT": xT,
            "wq": wt(wqa), "wk": wt(wka), "wv": wt(wva),
            "wp": wpa,
            "tri2": tri2,
            "zero2": zero2,
            "idin": ident,
            "vconst": vconst,
        })
    return in_maps


def kernel(x, w_attn, w_proj):
    from concourse.bass_utils import run_bass_kernel_spmd

    x = np.asarray(x, dtype=np.float32)
    w_attn = np.asarray(w_attn, dtype=np.float32)
    w_proj = np.asarray(w_proj, dtype=np.float32)

    if "nc" not in _CACHE:
        _CACHE["nc"] = build_program()
    nc = _CACHE["nc"]

    in_maps = _prepare_inputs(x, w_attn, w_proj)
    res = run_bass_kernel_spmd(nc, in_maps, list(range(NCORES)))
    acc = np.zeros((P, B * T // (4 * P), 4, C), dtype=np.float64)
    for r in res.results:
        acc += r["out"].astype(np.float64)
    # un-permute: out[(g*4+f)*128 + p, :] = acc[p, g, f, :]
    full = acc.transpose(1, 2, 0, 3).reshape(B * T, C)
    return full.reshape(B, T, C).astype(np.float32)



# revision 54
# speedup vs baseline: 1.4907x; 1.4907x over previous
"""Trainium2 Bass kernel for causal multi-head attention (B=4, T=2048, C=1024, H=16).

v4: batch x head-half sharding — core c handles batch c//2, heads
8*(c%2) .. 8*(c%2)+7 (column-parallel c_attn, row-parallel c_proj; the host
sums each batch's two partial projections). 4x less DMA than head-parallel.

Per-core dataflow (1 batch, 8 heads = 4 head-pairs, D2=512):
  - Phase A: QKV projection from host-pre-transposed fp16 x^T, emitted as
    per-quad thunks that are SPREAD between the attention j-steps of the
    previous q-chunk, so the PE fills ScalarE-bound stretches. Q^T/K^T kept
    [dims, tok]; V is projected directly in [tok, dims] layout (lhsT = x^T
    slice), so no PE transposes exist anywhere in the kernel.
  - v_sb per head holds [dims | ones] (even heads) or [ones | dims] (odd),
    so each AV matmul (M=128) emits both y^T AND the softmax denominator
    replicated 64x, in the partition half matching that head's y2t rows.
  - Phase B rounds (ci, pair): S^T[k, q] = K^T tile x Q^T per head; the two
    heads of a pair sit on disjoint PE row groups (K=64 at partitions 0-63 /
    64-127). exp is ONE ScalarE activation with a 3D AP covering both heads;
    causality streams only columns >= the 128-aligned diagonal (c0), and the
    diagonal 128-block gets a triangular-mask multiply (DVE).
  - Normalization: one DVE reciprocal per head (crosses partition halves)
    plus one fully partition-aligned DVE multiply straight into y2^T fp16 —
    no transposes, no gather/scatter.
  - Emission is software-pipelined with lookahead 3 (QK/exp of steps i+1..i+3
    precede AV of step i) so the in-order PE stream does not wait on ScalarE.
  - Out projection per ci: y2^T quads x w_proj halves in 4-step accumulation
    chains, emitted as per-token-tile thunks spread across the NEXT ci's
    j-steps (like the projection) so neither PE nor ScalarE sees a burst;
    fp16 output in a permuted tile-major layout (host un-permutes and sums
    each batch's core pair). PSUM drains are split DVE (k, v, out-proj) /
    ScalarE (q): the q copies land where ScalarE has slack (phase A), while
    out-proj drains stay off ScalarE because the late rounds are exp-saturated
    there and DVE idles; GPSIMD cannot touch PSUM and handles only DMA
    triggers.
"""

import sys
import numpy as np

sys.path.insert(0, "/opt/trn_rl_repo")

B, T, C = 4, 2048, 1024
H = 16
D = C // H            # 64
NCORES = 8
NH = 8                # heads per core
D2 = NH * D           # 512
NQ = 4                # 128-dim quads per core (quad qd = heads 2qd, 2qd+1)
NPAIR = 4
P = 128
KC = C // P           # 8 contraction tiles
PC = 512              # chunk tokens
QC = 512              # attention q chunk
NCH = T // PC         # 4
NT = T // P           # 16

_CACHE = {}


def build_program():
    import concourse.bacc as bacc
    import concourse.mybir as mybir
    from concourse import tile

    F32 = mybir.dt.float32
    F16 = mybir.dt.float16
    EXP = mybir.ActivationFunctionType.Exp

    nc = bacc.Bacc(None, target_bir_lowering=False, debug=False)

    xT = nc.declare_dram_parameter("xT", [NCH, P, KC, PC], F16, isOutput=False)
    wq = nc.declare_dram_parameter("wq", [P, KC, D2], F16, isOutput=False)
    wk = nc.declare_dram_parameter("wk", [P, KC, D2], F16, isOutput=False)
    wv = nc.declare_dram_parameter("wv", [P, KC, D2], F16, isOutput=False)
    wp = nc.declare_dram_parameter("wp", [P, NQ, C], F16, isOutput=False)
    tri2 = nc.declare_dram_parameter("tri2", [P, 2, P], F16, isOutput=False)
    # out[p, ci, f, :] = partial out row (ci*4+f)*128 + p (host un-permutes)
    out = nc.declare_dram_parameter("out", [P, 4, 4, C], F16, isOutput=True)

    with tile.TileContext(nc) as tc:
        with (
            tc.tile_pool(name="const", bufs=1) as const,
            tc.tile_pool(name="state", bufs=1) as state,
            tc.tile_pool(name="xtp", bufs=2) as xtp,
            tc.tile_pool(name="expp", bufs=7) as expp,
            tc.tile_pool(name="recp", bufs=3) as recp,
            tc.tile_pool(name="outp", bufs=4) as outp,
            tc.tile_pool(name="ps", bufs=3, space="PSUM") as ps,
        ):
            wq_sb = const.tile([P, KC, D2], F16, tag="wq")
            wk_sb = const.tile([P, KC, D2], F16, tag="wk")
            wv_sb = const.tile([P, KC, D2], F16, tag="wv")
            wp_sb = const.tile([P, NQ, C], F16, tag="wp")
            tri_sb = const.tile([P, 2, P], F16, tag="tri")

            qt_sb = state.tile([P, NQ, T], F16, tag="qt")
            kt_sb = state.tile([P, NQ, T], F16, tag="kt")
            # even heads: [dims | ones]; odd heads: [ones | dims] — so dims
            # land at the partition half matching their y2t rows and the
            # normalization muls are partition-aligned.
            v_sb = state.tile([P, NT, NH, 2, D], F16, tag="v")
            y2t_sb = state.tile([P, NQ, T], F16, tag="y2t")

            # spread const loads across queues so the first proj group (k)
            # can start after ~one DMA latency
            for piece in range(4):
                ksl = slice(2 * piece, 2 * piece + 2)
                nc.scalar.dma_start(out=wk_sb[:, ksl, :], in_=wk[:, ksl, :])
            nc.gpsimd.dma_start(out=wq_sb[:], in_=wq[:])
            nc.scalar.dma_start(out=tri_sb[:], in_=tri2[:])
            deferred_consts = [(wp_sb, wp)]
            wv_pending = [(wv_sb, wv)]  # issued on sync right after xT ch0
            nc.vector.memset(v_sb[:, :, 0:NH:2, 1, :], 1.0)
            nc.vector.memset(v_sb[:, :, 1:NH:2, 0, :], 1.0)

            def chunk_thunks(ch):
                """Phase A for chunk ch as a list of thunks (one per PE-group)
                so projection work can be spread between attention j-steps."""
                xt = xtp.tile([P, KC, PC], F16, tag="xt", name="xt")
                if ch == 0:
                    # split the first loads so the kc-chain can start after
                    # ~half a DMA latency (region-level deps)
                    for piece in range(4):
                        ksl = slice(2 * piece, 2 * piece + 2)
                        nc.sync.dma_start(out=xt[:, ksl, :],
                                          in_=xT[ch][:, ksl, :])
                else:
                    nc.sync.dma_start(out=xt[:], in_=xT[ch])
                if wv_pending:
                    dst, src = wv_pending.pop()
                    nc.sync.dma_start(out=dst[:], in_=src[:])
                if deferred_consts:
                    dst, src = deferred_consts.pop(0)
                    nc.scalar.dma_start(out=dst[:], in_=src[:])

                def proj_slot(which, w_sb, qd):
                    """One [128, 512] psum accumulation for q/k quad qd."""
                    pt = ps.tile([P, PC], F32, tag="ps", name="pt")
                    for kc in range(KC):
                        nc.tensor.matmul(
                            pt[:, :],
                            w_sb[:, kc, qd * P:(qd + 1) * P],
                            xt[:, kc, :],
                            start=(kc == 0), stop=(kc == KC - 1),
                        )
                    tsl = slice(ch * PC, (ch + 1) * PC)
                    if which == "q":
                        nc.scalar.copy(qt_sb[:, qd, tsl], pt[:, :])
                    else:
                        nc.vector.tensor_copy(kt_sb[:, qd, tsl], pt[:, :])

                def v_tile(tl):
                    """V for token-tile ch*4+tl directly as [tok, dims] via
                    lhsT = x^T slice — no transposes needed."""
                    pt = ps.tile([P, 4, P], F32, tag="ps", name="pv")
                    for kc in range(KC):
                        nc.tensor.matmul(
                            pt[:, :, :],
                            xt[:, kc, tl * P:(tl + 1) * P],
                            wv_sb[:, kc, :],
                            start=(kc == 0), stop=(kc == KC - 1),
                        )
                    tt = ch * 4 + tl
                    nc.vector.tensor_copy(
                        v_sb[:, tt, 0:NH:2, 0, :], pt[:, :, 0:D])
                    nc.vector.tensor_copy(
                        v_sb[:, tt, 1:NH:2, 1, :], pt[:, :, D:2 * D])

                import functools
                thunks = []
                # k first (QK of the next ci needs it soonest), then q, then v
                for which, w_sb in (("k", wk_sb), ("q", wq_sb)):
                    for qd in range(NQ):
                        thunks.append(
                            functools.partial(proj_slot, which, w_sb, qd))
                for tl in range(4):
                    thunks.append(functools.partial(v_tile, tl))
                return thunks

            def emit_qk(ci, pair, j, yt_unused):
                q0 = ci * QC
                c0 = max(0, P * j - q0)
                st = ps.tile([P, 2, QC], F32, tag="ps", name="st")
                for h2 in range(2):
                    rows = slice(h2 * D, h2 * D + D)
                    nc.tensor.matmul(
                        st[:, h2, c0:QC],
                        kt_sb[rows, pair, j * P:(j + 1) * P],
                        qt_sb[rows, pair, q0 + c0:q0 + QC],
                        start=True, stop=True,
                    )
                et = expp.tile([P, 2, QC], F16, tag="exp", name="et")
                nc.scalar.activation(
                    et[:, :, c0:QC], st[:, :, c0:QC], EXP,
                    scale=float(1.0 / np.sqrt(D)))
                if P * j >= q0:
                    nc.vector.tensor_mul(
                        et[:, :, c0:c0 + P], et[:, :, c0:c0 + P], tri_sb[:])
                return et

            def emit_av(ci, pair, j, njt, et, yt):
                q0 = ci * QC
                c0 = max(0, P * j - q0)
                for h2 in range(2):
                    h = 2 * pair + h2
                    nc.tensor.matmul(
                        yt[:, h2, c0:QC],
                        v_sb[:, j, h, :, :],
                        et[:, h2, c0:QC],
                        start=(j == 0), stop=(j == njt - 1),
                        skip_group_check=True,
                    )

            def emit_divides(ci, pair, yt):
                # yt slot 0 (even head): dims@0:64, denom@64:128; slot 1 (odd
                # head): denom@0:64, dims@64:128. Reciprocals cross partition
                # halves; the muls are fully partition-aligned.
                q0 = ci * QC
                rec = recp.tile([P, QC], F32, tag="rec", name="rec")
                nc.vector.reciprocal(rec[0:D, :], yt[D:2 * D, 0, :])
                nc.vector.reciprocal(rec[D:2 * D, :], yt[0:D, 1, :])
                nc.vector.tensor_mul(
                    y2t_sb[0:D, pair, q0:q0 + QC], yt[0:D, 0, :], rec[0:D, :])
                nc.vector.tensor_mul(
                    y2t_sb[D:2 * D, pair, q0:q0 + QC], yt[D:2 * D, 1, :],
                    rec[D:2 * D, :])

            def outproj_thunks(ci):
                """Out-projection for ci as 4 per-token-tile thunks so its PE
                and ScalarE work spreads across the next ci's j-steps."""
                import functools
                state = {}

                def tt_thunk(tt2, f2):
                    if f2 == 0:
                        state[tt2] = outp.tile([P, 2, C], F16, tag="osb",
                                               name="osb")
                    osb = state[tt2]
                    tt = ci * 4 + tt2 * 2 + f2
                    pps = ps.tile([P, 2, QC], F32, tag="ps", name="pps")
                    for s in range(2):
                        for qd in range(NQ):
                            nc.tensor.matmul(
                                pps[:, s, :],
                                y2t_sb[:, qd, tt * P:(tt + 1) * P],
                                wp_sb[:, qd, s * QC:(s + 1) * QC],
                                start=(qd == 0), stop=(qd == NQ - 1),
                            )
                    nc.vector.tensor_copy(osb[:, f2, :], pps[:])
                    nc.gpsimd.dma_start(
                        out=out[:, ci, tt2 * 2 + f2:tt2 * 2 + f2 + 1, :],
                        in_=osb[:, f2:f2 + 1, :])

                return [functools.partial(tt_thunk, tt2, f2)
                        for tt2 in range(2) for f2 in range(2)]

            # ---- software-pipelined emission (lookahead 2) ----
            # chunk 0 emitted up front; chunk ci+1's groups are spread across
            # ci's j-steps so PE fills the ACT-bound attention stretches.
            from collections import deque
            pipe = deque()       # pending (ci, pair, j, njt, et, yt)

            extra = []           # spreadable out-projection thunks

            def flush_one():
                if not pipe:
                    return
                pci, ppair, pj, pnjt, pet, pyt = pipe.popleft()
                emit_av(pci, ppair, pj, pnjt, pet, pyt)
                if pj == pnjt - 1:
                    emit_divides(pci, ppair, pyt)
                    if ppair == NPAIR - 1:
                        extra.extend(outproj_thunks(pci))

            # chunk 0: run only what round (0, pair 0) needs up front
            # (k/q quad 0 and the V tiles); spread the other quads into
            # ci=0's j-steps so ScalarE starts exps ~10us earlier.
            ch0 = chunk_thunks(0)
            for idx in (0, 4, 8, 9, 10, 11):
                ch0[idx]()
            rest0 = [ch0[i] for i in (1, 5, 2, 6, 3, 7)]  # k/q quads 1..3
            pending = []
            for ci in range(NCH):
                if ci + 1 < NCH:
                    pending = (rest0 if ci == 0 else []) + chunk_thunks(ci + 1)
                n_steps = 16 * (ci + 1)
                n_thunks0 = len(pending)
                popped = 0
                step = 0
                for pair in range(NPAIR):
                    njt = 4 * (ci + 1)
                    yt = ps.tile([P, 2, QC], F32, tag="yt", bufs=1,
                                 name="yt")
                    for j in range(njt):
                        et = emit_qk(ci, pair, j, yt)
                        pipe.append((ci, pair, j, njt, et, yt))
                        if len(pipe) > 3:
                            flush_one()
                        step += 1
                        took = False
                        while (pending and
                               step * n_thunks0 >= (popped + 1) * n_steps):
                            pending.pop(0)()
                            popped += 1
                            took = True
                        if not took and extra:
                            extra.pop(0)()
                while pending:
                    pending.pop(0)()
            while pipe:
                flush_one()
            while extra:
                extra.pop(0)()

    nc.compile()
    return nc


def _prepare_inputs(x, w_attn, w_proj):
    # xT_b[ch, p, kc, t] = x[b, ch*PC + t, kc*128 + p]
    xT_all = [
        np.ascontiguousarray(
            x[b].reshape(NCH, PC, KC, P).transpose(0, 3, 2, 1)
        ).astype(np.float16)
        for b in range(B)
    ]

    kk = np.arange(P)[:, None]
    qq = np.arange(P)[None, :]
    tri = (qq >= kk).astype(np.float16)
    tri2 = np.ascontiguousarray(np.broadcast_to(tri[:, None, :], (P, 2, P)))
    ident = np.eye(P, dtype=np.float16)

    def wt(w):
        return np.ascontiguousarray(
            w.reshape(KC, P, D2).transpose(1, 0, 2)).astype(np.float16)

    in_maps = []
    for c in range(NCORES):
        b, hh = c // 2, c % 2
        cols = slice(hh * D2, (hh + 1) * D2)
        wqa = wt(w_attn[:, 0 * C:1 * C][:, cols])
        wka = wt(w_attn[:, 1 * C:2 * C][:, cols])
        wva = wt(w_attn[:, 2 * C:3 * C][:, cols])
        wpa = np.ascontiguousarray(
            w_proj[hh * D2:(hh + 1) * D2, :].reshape(NQ, P, C)
            .transpose(1, 0, 2)).astype(np.float16)
        in_maps.append({
            "xT": xT_all[b],
            "wq": wqa, "wk": wka, "wv": wva,
            "wp": wpa,
            "tri2": tri2,
        })
    return in_maps


def combine_outputs(outs):
    """outs: list of 8 per-core 'out' arrays [P, 4, 4, C] -> full [B, T, C]."""
    full = np.empty((B, T, C), dtype=np.float32)
    for b in range(B):
        acc = outs[2 * b].astype(np.float32) + outs[2 * b + 1].astype(np.float32)
        full[b] = acc.transpose(1, 2, 0, 3).reshape(T, C)
    return full


def kernel(x, w_attn, w_proj):
    from concourse.bass_utils import run_bass_kernel_spmd

    x = np.asarray(x, dtype=np.float32)
    w_attn = np.asarray(w_attn, dtype=np.float32)
    w_proj = np.asarray(w_proj, dtype=np.float32)

    if "nc" not in _CACHE:
        _CACHE["nc"] = build_program()
    nc = _CACHE["nc"]

    in_maps = _prepare_inputs(x, w_attn, w_proj)
    res = run_bass_kernel_spmd(nc, in_maps, list(range(NCORES)))
    return combine_outputs([r["out"] for r in res.results])


# revision 57
# speedup vs baseline: 1.8851x; 1.2646x over previous
"""Trainium2 Bass kernel for causal multi-head attention (B=4, T=2048, C=1024, H=16).

v4: batch x head-half sharding — core c handles batch c//2, heads
8*(c%2) .. 8*(c%2)+7 (column-parallel c_attn, row-parallel c_proj; the host
sums each batch's two partial projections). 4x less DMA than head-parallel.

Per-core dataflow (1 batch, 8 heads = 4 head-pairs, D2=512):
  - Phase A: QKV projection from host-pre-transposed fp16 x^T, emitted as
    per-quad thunks that are SPREAD between the attention j-steps of the
    previous q-chunk, so the PE fills ScalarE-bound stretches. Q^T/K^T kept
    [dims, tok]; V is projected directly in [tok, dims] layout (lhsT = x^T
    slice), so no PE transposes exist anywhere in the kernel.
  - v_sb per head holds [dims | ones] (even heads) or [ones | dims] (odd),
    so each AV matmul (M=128) emits both y^T AND the softmax denominator
    replicated 64x, in the partition half matching that head's y2t rows.
  - Phase B rounds (ci, pair): S^T[k, q] = K^T tile x Q^T per head; the two
    heads of a pair sit on disjoint PE row groups (K=64 at partitions 0-63 /
    64-127). exp is ONE ScalarE activation with a 3D AP covering both heads;
    causality streams only columns >= the 128-aligned diagonal (c0), and the
    diagonal 128-block gets a triangular-mask multiply (DVE).
  - Normalization: one DVE reciprocal per head (crosses partition halves)
    plus one fully partition-aligned DVE multiply straight into y2^T fp16 —
    no transposes, no gather/scatter.
  - Emission is software-pipelined with lookahead 3 (QK/exp of steps i+1..i+3
    precede AV of step i) so the in-order PE stream does not wait on ScalarE.
  - Out projection per ci: y2^T quads x w_proj halves in 4-step accumulation
    chains, emitted as per-token-tile thunks spread across the NEXT ci's
    j-steps (like the projection) so neither PE nor ScalarE sees a burst;
    fp16 output in a permuted tile-major layout (host un-permutes and sums
    each batch's core pair). PSUM drains are split DVE (k, v, out-proj) /
    ScalarE (q): the q copies land where ScalarE has slack (phase A), while
    out-proj drains stay off ScalarE because the late rounds are exp-saturated
    there and DVE idles; GPSIMD cannot touch PSUM and handles only DMA
    triggers.
"""

import sys
import numpy as np

sys.path.insert(0, "/opt/trn_rl_repo")

B, T, C = 4, 2048, 1024
H = 16
D = C // H            # 64
NCORES = 8
NH = 8                # heads per core
D2 = NH * D           # 512
NQ = 4                # 128-dim quads per core (quad qd = heads 2qd, 2qd+1)
NPAIR = 4
P = 128
KC = C // P           # 8 contraction tiles
PC = 512              # chunk tokens
QC = 512              # attention q chunk
NCH = T // PC         # 4
NT = T // P           # 16

_CACHE = {}


def build_program():
    import concourse.bacc as bacc
    import concourse.mybir as mybir
    from concourse import tile

    F32 = mybir.dt.float32
    F16 = mybir.dt.float16
    EXP = mybir.ActivationFunctionType.Exp

    nc = bacc.Bacc(None, target_bir_lowering=False, debug=False)

    xT = nc.declare_dram_parameter("xT", [NCH, P, KC, PC], F16, isOutput=False)
    wq = nc.declare_dram_parameter("wq", [P, KC, D2], F16, isOutput=False)
    wk = nc.declare_dram_parameter("wk", [P, KC, D2], F16, isOutput=False)
    wv = nc.declare_dram_parameter("wv", [P, KC, D2], F16, isOutput=False)
    wp = nc.declare_dram_parameter("wp", [P, NQ, C], F16, isOutput=False)
    tri2 = nc.declare_dram_parameter("tri2", [P, 2, P], F16, isOutput=False)
    # out[p, ci, f, :] = partial out row (ci*4+f)*128 + p (host un-permutes)
    out = nc.declare_dram_parameter("out", [P, 4, 4, C], F16, isOutput=True)

    with tile.TileContext(nc) as tc:
        with (
            tc.tile_pool(name="const", bufs=1) as const,
            tc.tile_pool(name="state", bufs=1) as state,
            tc.tile_pool(name="xtp", bufs=2) as xtp,
            tc.tile_pool(name="expp", bufs=7) as expp,
            tc.tile_pool(name="recp", bufs=3) as recp,
            tc.tile_pool(name="outp", bufs=4) as outp,
            tc.tile_pool(name="ps", bufs=3, space="PSUM") as ps,
        ):
            wq_sb = const.tile([P, KC, D2], F16, tag="wq")
            wk_sb = const.tile([P, KC, D2], F16, tag="wk")
            wv_sb = const.tile([P, KC, D2], F16, tag="wv")
            wp_sb = const.tile([P, NQ, C], F16, tag="wp")
            tri_sb = const.tile([P, 2, P], F16, tag="tri")

            qt_sb = state.tile([P, NQ, T], F16, tag="qt")
            kt_sb = state.tile([P, NQ, T], F16, tag="kt")
            # even heads: [dims | ones]; odd heads: [ones | dims] — so dims
            # land at the partition half matching their y2t rows and the
            # normalization muls are partition-aligned.
            v_sb = state.tile([P, NT, NH, 2, D], F16, tag="v")
            y2t_sb = state.tile([P, NQ, T], F16, tag="y2t")

            # spread const loads across queues so the first proj group (k)
            # can start after ~one DMA latency
            for piece in range(4):
                ksl = slice(2 * piece, 2 * piece + 2)
                nc.scalar.dma_start(out=wk_sb[:, ksl, :], in_=wk[:, ksl, :])
            nc.gpsimd.dma_start(out=wq_sb[:], in_=wq[:])
            nc.scalar.dma_start(out=tri_sb[:], in_=tri2[:])
            deferred_consts = [(wp_sb, wp)]
            wv_pending = [(wv_sb, wv)]  # issued on sync right after xT ch0
            nc.vector.memset(v_sb[:, :, 0:NH:2, 1, :], 1.0)
            nc.vector.memset(v_sb[:, :, 1:NH:2, 0, :], 1.0)

            def chunk_thunks(ch):
                """Phase A for chunk ch as a list of thunks (one per PE-group)
                so projection work can be spread between attention j-steps."""
                xt = xtp.tile([P, KC, PC], F16, tag="xt", name="xt")
                if ch == 0:
                    # split the first loads so the kc-chain can start after
                    # ~half a DMA latency (region-level deps)
                    for piece in range(4):
                        ksl = slice(2 * piece, 2 * piece + 2)
                        nc.sync.dma_start(out=xt[:, ksl, :],
                                          in_=xT[ch][:, ksl, :])
                else:
                    nc.sync.dma_start(out=xt[:], in_=xT[ch])
                if wv_pending:
                    dst, src = wv_pending.pop()
                    nc.sync.dma_start(out=dst[:], in_=src[:])
                if deferred_consts:
                    dst, src = deferred_consts.pop(0)
                    nc.scalar.dma_start(out=dst[:], in_=src[:])

                def proj_slot(which, w_sb, qd):
                    """One [128, 512] psum accumulation for q/k quad qd."""
                    pt = ps.tile([P, PC], F32, tag="ps", name="pt")
                    for kc in range(KC):
                        nc.tensor.matmul(
                            pt[:, :],
                            w_sb[:, kc, qd * P:(qd + 1) * P],
                            xt[:, kc, :],
                            start=(kc == 0), stop=(kc == KC - 1),
                        )
                    tsl = slice(ch * PC, (ch + 1) * PC)
                    if which == "q":
                        nc.scalar.copy(qt_sb[:, qd, tsl], pt[:, :])
                    else:
                        nc.vector.tensor_copy(kt_sb[:, qd, tsl], pt[:, :])

                def v_tile(tl):
                    """V for token-tile ch*4+tl directly as [tok, dims] via
                    lhsT = x^T slice — no transposes needed."""
                    pt = ps.tile([P, 4, P], F32, tag="ps", name="pv")
                    for kc in range(KC):
                        nc.tensor.matmul(
                            pt[:, :, :],
                            xt[:, kc, tl * P:(tl + 1) * P],
                            wv_sb[:, kc, :],
                            start=(kc == 0), stop=(kc == KC - 1),
                        )
                    tt = ch * 4 + tl
                    nc.vector.tensor_copy(
                        v_sb[:, tt, 0:NH:2, 0, :], pt[:, :, 0:D])
                    nc.vector.tensor_copy(
                        v_sb[:, tt, 1:NH:2, 1, :], pt[:, :, D:2 * D])

                import functools
                thunks = []
                # k first (QK of the next ci needs it soonest), then q, then v
                for which, w_sb in (("k", wk_sb), ("q", wq_sb)):
                    for qd in range(NQ):
                        thunks.append(
                            functools.partial(proj_slot, which, w_sb, qd))
                for tl in range(4):
                    thunks.append(functools.partial(v_tile, tl))
                return thunks

            def emit_qk(ci, pair, j, yt_unused):
                q0 = ci * QC
                c0 = max(0, P * j - q0)
                st = ps.tile([P, 2, QC], F32, tag="ps", name="st")
                for h2 in range(2):
                    rows = slice(h2 * D, h2 * D + D)
                    nc.tensor.matmul(
                        st[:, h2, c0:QC],
                        kt_sb[rows, pair, j * P:(j + 1) * P],
                        qt_sb[rows, pair, q0 + c0:q0 + QC],
                        start=True, stop=True,
                    )
                et = expp.tile([P, 2, QC], F16, tag="exp", name="et")
                nc.scalar.activation(
                    et[:, :, c0:QC], st[:, :, c0:QC], EXP,
                    scale=float(1.0 / np.sqrt(D)))
                if P * j >= q0:
                    nc.vector.tensor_mul(
                        et[:, :, c0:c0 + P], et[:, :, c0:c0 + P], tri_sb[:])
                return et

            def emit_av(ci, pair, j, njt, et, yt):
                q0 = ci * QC
                c0 = max(0, P * j - q0)
                for h2 in range(2):
                    h = 2 * pair + h2
                    nc.tensor.matmul(
                        yt[:, h2, c0:QC],
                        v_sb[:, j, h, :, :],
                        et[:, h2, c0:QC],
                        start=(j == 0), stop=(j == njt - 1),
                        skip_group_check=True,
                    )

            def emit_divides(ci, pair, yt):
                # yt slot 0 (even head): dims@0:64, denom@64:128; slot 1 (odd
                # head): denom@0:64, dims@64:128. Reciprocals cross partition
                # halves; the muls are fully partition-aligned.
                q0 = ci * QC
                rec = recp.tile([P, QC], F32, tag="rec", name="rec")
                nc.vector.reciprocal(rec[0:D, :], yt[D:2 * D, 0, :])
                nc.vector.reciprocal(rec[D:2 * D, :], yt[0:D, 1, :])
                nc.vector.tensor_mul(
                    y2t_sb[0:D, pair, q0:q0 + QC], yt[0:D, 0, :], rec[0:D, :])
                nc.vector.tensor_mul(
                    y2t_sb[D:2 * D, pair, q0:q0 + QC], yt[D:2 * D, 1, :],
                    rec[D:2 * D, :])

            def outproj_thunks(ci):
                """Out-projection for ci as 4 per-token-tile thunks so its PE
                and ScalarE work spreads across the next ci's j-steps."""
                import functools
                state = {}

                def tt_thunk(tt2, f2):
                    if f2 == 0:
                        state[tt2] = outp.tile([P, 2, C], F16, tag="osb",
                                               name="osb")
                    osb = state[tt2]
                    tt = ci * 4 + tt2 * 2 + f2
                    pps = ps.tile([P, 2, QC], F32, tag="ps", name="pps")
                    for s in range(2):
                        for qd in range(NQ):
                            nc.tensor.matmul(
                                pps[:, s, :],
                                y2t_sb[:, qd, tt * P:(tt + 1) * P],
                                wp_sb[:, qd, s * QC:(s + 1) * QC],
                                start=(qd == 0), stop=(qd == NQ - 1),
                            )
                    nc.vector.tensor_copy(osb[:, f2, :], pps[:])
                    nc.gpsimd.dma_start(
                        out=out[:, ci, tt2 * 2 + f2:tt2 * 2 + f2 + 1, :],
                        in_=osb[:, f2:f2 + 1, :])

                return [functools.partial(tt_thunk, tt2, f2)
                        for tt2 in range(2) for f2 in range(2)]

            # ---- software-pipelined emission (lookahead 2) ----
            # chunk 0 emitted up front; chunk ci+1's groups are spread across
            # ci's j-steps so PE fills the ACT-bound attention stretches.
            from collections import deque
            pipe = deque()       # pending (ci, pair, j, njt, et, yt)

            extra = []           # spreadable out-projection thunks

            def flush_one():
                if not pipe:
                    return
                pci, ppair, pj, pnjt, pet, pyt = pipe.popleft()
                emit_av(pci, ppair, pj, pnjt, pet, pyt)
                if pj == pnjt - 1:
                    emit_divides(pci, ppair, pyt)
                    if ppair == NPAIR - 1:
                        extra.extend(outproj_thunks(pci))

            # chunk 0: run only what round (0, pair 0) needs up front
            # (k/q quad 0 and the V tiles); spread the other quads into
            # ci=0's j-steps so ScalarE starts exps ~10us earlier.
            ch0 = chunk_thunks(0)
            for idx in (0, 4, 8, 9):
                ch0[idx]()
            # v2/v3 join the spread queue (needed only by AV j=2,3, several
            # steps in); k/q quads 1..3 follow in pair order
            rest0 = [ch0[i] for i in (10, 11, 1, 5, 2, 6, 3, 7)]
            pending = []
            for ci in range(NCH):
                if ci + 1 < NCH:
                    pending = (rest0 if ci == 0 else []) + chunk_thunks(ci + 1)
                n_steps = 16 * (ci + 1)
                n_thunks0 = len(pending)
                popped = 0
                step = 0
                for pair in range(NPAIR):
                    njt = 4 * (ci + 1)
                    yt = ps.tile([P, 2, QC], F32, tag="yt", bufs=1,
                                 name="yt")
                    for j in range(njt):
                        et = emit_qk(ci, pair, j, yt)
                        pipe.append((ci, pair, j, njt, et, yt))
                        if len(pipe) > 3:
                            flush_one()
                        step += 1
                        took = False
                        while (pending and
                               step * n_thunks0 >= (popped + 1) * n_steps):
                            pending.pop(0)()
                            popped += 1
                            took = True
                        if not took and extra:
                            extra.pop(0)()
                while pending:
                    pending.pop(0)()
            while pipe:
                flush_one()
            while extra:
                extra.pop(0)()

    nc.compile()
    return nc


def _prepare_inputs(x, w_attn, w_proj):
    # xT_b[ch, p, kc, t] = x[b, ch*PC + t, kc*128 + p]
    xT_all = [
        np.ascontiguousarray(
            x[b].reshape(NCH, PC, KC, P).transpose(0, 3, 2, 1)
        ).astype(np.float16)
        for b in range(B)
    ]

    kk = np.arange(P)[:, None]
    qq = np.arange(P)[None, :]
    tri = (qq >= kk).astype(np.float16)
    tri2 = np.ascontiguousarray(np.broadcast_to(tri[:, None, :], (P, 2, P)))
    ident = np.eye(P, dtype=np.float16)

    def wt(w):
        return np.ascontiguousarray(
            w.reshape(KC, P, D2).transpose(1, 0, 2)).astype(np.float16)

    in_maps = []
    for c in range(NCORES):
        b, hh = c // 2, c % 2
        cols = slice(hh * D2, (hh + 1) * D2)
        wqa = wt(w_attn[:, 0 * C:1 * C][:, cols])
        wka = wt(w_attn[:, 1 * C:2 * C][:, cols])
        wva = wt(w_attn[:, 2 * C:3 * C][:, cols])
        wpa = np.ascontiguousarray(
            w_proj[hh * D2:(hh + 1) * D2, :].reshape(NQ, P, C)
            .transpose(1, 0, 2)).astype(np.float16)
        in_maps.append({
            "xT": xT_all[b],
            "wq": wqa, "wk": wka, "wv": wva,
            "wp": wpa,
            "tri2": tri2,
        })
    return in_maps


def combine_outputs(outs):
    """outs: list of 8 per-core 'out' arrays [P, 4, 4, C] -> full [B, T, C]."""
    full = np.empty((B, T, C), dtype=np.float32)
    for b in range(B):
        acc = outs[2 * b].astype(np.float32) + outs[2 * b + 1].astype(np.float32)
        full[b] = acc.transpose(1, 2, 0, 3).reshape(T, C)
    return full


def kernel(x, w_attn, w_proj):
    from concourse.bass_utils import run_bass_kernel_spmd

    x = np.asarray(x, dtype=np.float32)
    w_attn = np.asarray(w_attn, dtype=np.float32)
    w_proj = np.asarray(w_proj, dtype=np.float32)

    if "nc" not in _CACHE:
        _CACHE["nc"] = build_program()
    nc = _CACHE["nc"]

    in_maps = _prepare_inputs(x, w_attn, w_proj)
    res = run_bass_kernel_spmd(nc, in_maps, list(range(NCORES)))
    return combine_outputs([r["out"] for r in res.results])


# revision 58
# speedup vs baseline: 1.9660x; 1.0429x over previous
"""Trainium2 Bass kernel for causal multi-head attention (B=4, T=2048, C=1024, H=16).

v4: batch x head-half sharding — core c handles batch c//2, heads
8*(c%2) .. 8*(c%2)+7 (column-parallel c_attn, row-parallel c_proj; the host
sums each batch's two partial projections). 4x less DMA than head-parallel.

Per-core dataflow (1 batch, 8 heads = 4 head-pairs, D2=512):
  - Phase A: QKV projection from host-pre-transposed fp16 x^T, emitted as
    per-quad thunks that are SPREAD between the attention j-steps of the
    previous q-chunk, so the PE fills ScalarE-bound stretches. Q^T/K^T kept
    [dims, tok]; V is projected directly in [tok, dims] layout (lhsT = x^T
    slice), so no PE transposes exist anywhere in the kernel.
  - v_sb per head holds [dims | ones] (even heads) or [ones | dims] (odd),
    so each AV matmul (M=128) emits both y^T AND the softmax denominator
    replicated 64x, in the partition half matching that head's y2t rows.
  - Phase B rounds (ci, pair): S^T[k, q] = K^T tile x Q^T per head; the two
    heads of a pair sit on disjoint PE row groups (K=64 at partitions 0-63 /
    64-127). exp is ONE ScalarE activation with a 3D AP covering both heads;
    causality streams only columns >= the 128-aligned diagonal (c0), and the
    diagonal 128-block gets a triangular-mask multiply (DVE).
  - Normalization: one DVE reciprocal per head (crosses partition halves)
    plus one fully partition-aligned DVE multiply straight into y2^T fp16 —
    no transposes, no gather/scatter.
  - Emission is software-pipelined with lookahead 3 (QK/exp of steps i+1..i+3
    precede AV of step i) so the in-order PE stream does not wait on ScalarE.
  - Out projection per ci: y2^T quads x w_proj halves in 4-step accumulation
    chains, emitted as per-token-tile thunks spread across the NEXT ci's
    j-steps (like the projection) so neither PE nor ScalarE sees a burst;
    fp16 output in a permuted tile-major layout (host un-permutes and sums
    each batch's core pair). PSUM drains are split DVE (k, v, out-proj) /
    ScalarE (q): the q copies land where ScalarE has slack (phase A), while
    out-proj drains stay off ScalarE because the late rounds are exp-saturated
    there and DVE idles; GPSIMD cannot touch PSUM and handles only DMA
    triggers.
"""

import sys
import numpy as np

sys.path.insert(0, "/opt/trn_rl_repo")

B, T, C = 4, 2048, 1024
H = 16
D = C // H            # 64
NCORES = 8
NH = 8                # heads per core
D2 = NH * D           # 512
NQ = 4                # 128-dim quads per core (quad qd = heads 2qd, 2qd+1)
NPAIR = 4
P = 128
KC = C // P           # 8 contraction tiles
PC = 512              # chunk tokens
QC = 512              # attention q chunk
NCH = T // PC         # 4
NT = T // P           # 16

_CACHE = {}


def build_program():
    import concourse.bacc as bacc
    import concourse.mybir as mybir
    from concourse import tile

    F32 = mybir.dt.float32
    F16 = mybir.dt.float16
    EXP = mybir.ActivationFunctionType.Exp

    nc = bacc.Bacc(None, target_bir_lowering=False, debug=False)

    xT = nc.declare_dram_parameter("xT", [NCH, P, KC, PC], F16, isOutput=False)
    wq = nc.declare_dram_parameter("wq", [P, KC, D2], F16, isOutput=False)
    wk = nc.declare_dram_parameter("wk", [P, KC, D2], F16, isOutput=False)
    wv = nc.declare_dram_parameter("wv", [P, KC, D2], F16, isOutput=False)
    wp = nc.declare_dram_parameter("wp", [P, NQ, C], F16, isOutput=False)
    tri2 = nc.declare_dram_parameter("tri2", [P, 2, P], F16, isOutput=False)
    # out[p, ci, f, :] = partial out row (ci*4+f)*128 + p (host un-permutes)
    out = nc.declare_dram_parameter("out", [P, 4, 4, C], F16, isOutput=True)

    with tile.TileContext(nc) as tc:
        with (
            tc.tile_pool(name="const", bufs=1) as const,
            tc.tile_pool(name="state", bufs=1) as state,
            tc.tile_pool(name="xtp", bufs=2) as xtp,
            tc.tile_pool(name="expp", bufs=7) as expp,
            tc.tile_pool(name="recp", bufs=3) as recp,
            tc.tile_pool(name="outp", bufs=4) as outp,
            tc.tile_pool(name="ps", bufs=3, space="PSUM") as ps,
        ):
            wq_sb = const.tile([P, KC, D2], F16, tag="wq")
            wk_sb = const.tile([P, KC, D2], F16, tag="wk")
            wv_sb = const.tile([P, KC, D2], F16, tag="wv")
            wp_sb = const.tile([P, NQ, C], F16, tag="wp")
            tri_sb = const.tile([P, 2, P], F16, tag="tri")

            qt_sb = state.tile([P, NQ, T], F16, tag="qt")
            kt_sb = state.tile([P, NQ, T], F16, tag="kt")
            # even heads: [dims | ones]; odd heads: [ones | dims] — so dims
            # land at the partition half matching their y2t rows and the
            # normalization muls are partition-aligned.
            v_sb = state.tile([P, NT, NH, 2, D], F16, tag="v")
            y2t_sb = state.tile([P, NQ, T], F16, tag="y2t")

            # spread const loads across queues so the first proj group (k)
            # can start after ~one DMA latency
            for piece in range(4):
                ksl = slice(2 * piece, 2 * piece + 2)
                nc.scalar.dma_start(out=wk_sb[:, ksl, :], in_=wk[:, ksl, :])
            nc.gpsimd.dma_start(out=wq_sb[:], in_=wq[:])
            nc.scalar.dma_start(out=tri_sb[:], in_=tri2[:])
            deferred_consts = [(wp_sb, wp)]
            wv_pending = [(wv_sb, wv)]  # issued on sync right after xT ch0
            nc.vector.memset(v_sb[:, :, 0:NH:2, 1, :], 1.0)
            nc.vector.memset(v_sb[:, :, 1:NH:2, 0, :], 1.0)

            def chunk_thunks(ch):
                """Phase A for chunk ch as a list of thunks (one per PE-group)
                so projection work can be spread between attention j-steps."""
                xt = xtp.tile([P, KC, PC], F16, tag="xt", name="xt")
                if ch == 0:
                    # split the first loads so the kc-chain can start after
                    # ~half a DMA latency (region-level deps)
                    for piece in range(4):
                        ksl = slice(2 * piece, 2 * piece + 2)
                        nc.sync.dma_start(out=xt[:, ksl, :],
                                          in_=xT[ch][:, ksl, :])
                else:
                    nc.sync.dma_start(out=xt[:], in_=xT[ch])
                if wv_pending:
                    dst, src = wv_pending.pop()
                    nc.sync.dma_start(out=dst[:], in_=src[:])
                if deferred_consts:
                    dst, src = deferred_consts.pop(0)
                    nc.scalar.dma_start(out=dst[:], in_=src[:])

                def proj_slot(which, w_sb, qd):
                    """One [128, 512] psum accumulation for q/k quad qd."""
                    pt = ps.tile([P, PC], F32, tag="ps", name="pt")
                    for kc in range(KC):
                        nc.tensor.matmul(
                            pt[:, :],
                            w_sb[:, kc, qd * P:(qd + 1) * P],
                            xt[:, kc, :],
                            start=(kc == 0), stop=(kc == KC - 1),
                        )
                    tsl = slice(ch * PC, (ch + 1) * PC)
                    if which == "q":
                        nc.scalar.copy(qt_sb[:, qd, tsl], pt[:, :])
                    else:
                        nc.vector.tensor_copy(kt_sb[:, qd, tsl], pt[:, :])

                def v_tile(tl):
                    """V for token-tile ch*4+tl directly as [tok, dims] via
                    lhsT = x^T slice — no transposes needed."""
                    pt = ps.tile([P, 4, P], F32, tag="ps", name="pv")
                    for kc in range(KC):
                        nc.tensor.matmul(
                            pt[:, :, :],
                            xt[:, kc, tl * P:(tl + 1) * P],
                            wv_sb[:, kc, :],
                            start=(kc == 0), stop=(kc == KC - 1),
                        )
                    tt = ch * 4 + tl
                    nc.vector.tensor_copy(
                        v_sb[:, tt, 0:NH:2, 0, :], pt[:, :, 0:D])
                    nc.vector.tensor_copy(
                        v_sb[:, tt, 1:NH:2, 1, :], pt[:, :, D:2 * D])

                import functools
                thunks = []
                # k first (QK of the next ci needs it soonest), then q, then v
                for which, w_sb in (("k", wk_sb), ("q", wq_sb)):
                    for qd in range(NQ):
                        thunks.append(
                            functools.partial(proj_slot, which, w_sb, qd))
                for tl in range(4):
                    thunks.append(functools.partial(v_tile, tl))
                return thunks

            def emit_qk(ci, pair, j, yt_unused):
                q0 = ci * QC
                c0 = max(0, P * j - q0)
                st = ps.tile([P, 2, QC], F32, tag="ps", name="st")
                for h2 in range(2):
                    rows = slice(h2 * D, h2 * D + D)
                    nc.tensor.matmul(
                        st[:, h2, c0:QC],
                        kt_sb[rows, pair, j * P:(j + 1) * P],
                        qt_sb[rows, pair, q0 + c0:q0 + QC],
                        start=True, stop=True,
                    )
                et = expp.tile([P, 2, QC], F16, tag="exp", name="et")
                nc.scalar.activation(
                    et[:, :, c0:QC], st[:, :, c0:QC], EXP,
                    scale=float(1.0 / np.sqrt(D)))
                if P * j >= q0:
                    nc.vector.tensor_mul(
                        et[:, :, c0:c0 + P], et[:, :, c0:c0 + P], tri_sb[:])
                return et

            def emit_av(ci, pair, j, njt, et, yt):
                q0 = ci * QC
                c0 = max(0, P * j - q0)
                for h2 in range(2):
                    h = 2 * pair + h2
                    nc.tensor.matmul(
                        yt[:, h2, c0:QC],
                        v_sb[:, j, h, :, :],
                        et[:, h2, c0:QC],
                        start=(j == 0), stop=(j == njt - 1),
                        skip_group_check=True,
                    )

            def emit_divides(ci, pair, yt):
                # yt slot 0 (even head): dims@0:64, denom@64:128; slot 1 (odd
                # head): denom@0:64, dims@64:128. Reciprocals cross partition
                # halves; the muls are fully partition-aligned.
                q0 = ci * QC
                rec = recp.tile([P, QC], F32, tag="rec", name="rec")
                nc.vector.reciprocal(rec[0:D, :], yt[D:2 * D, 0, :])
                nc.vector.reciprocal(rec[D:2 * D, :], yt[0:D, 1, :])
                nc.vector.tensor_mul(
                    y2t_sb[0:D, pair, q0:q0 + QC], yt[0:D, 0, :], rec[0:D, :])
                nc.vector.tensor_mul(
                    y2t_sb[D:2 * D, pair, q0:q0 + QC], yt[D:2 * D, 1, :],
                    rec[D:2 * D, :])

            def outproj_thunks(ci):
                """Out-projection for ci as 4 per-token-tile thunks so its PE
                and ScalarE work spreads across the next ci's j-steps."""
                import functools
                state = {}

                def tt_thunk(tt2, f2):
                    if f2 == 0:
                        state[tt2] = outp.tile([P, 2, C], F16, tag="osb",
                                               name="osb")
                    osb = state[tt2]
                    tt = ci * 4 + tt2 * 2 + f2
                    pps = ps.tile([P, 2, QC], F32, tag="ps", name="pps")
                    for s in range(2):
                        for qd in range(NQ):
                            nc.tensor.matmul(
                                pps[:, s, :],
                                y2t_sb[:, qd, tt * P:(tt + 1) * P],
                                wp_sb[:, qd, s * QC:(s + 1) * QC],
                                start=(qd == 0), stop=(qd == NQ - 1),
                            )
                    nc.vector.tensor_copy(osb[:, f2, :], pps[:])
                    nc.gpsimd.dma_start(
                        out=out[:, ci, tt2 * 2 + f2:tt2 * 2 + f2 + 1, :],
                        in_=osb[:, f2:f2 + 1, :])

                return [functools.partial(tt_thunk, tt2, f2)
                        for tt2 in range(2) for f2 in range(2)]

            # ---- software-pipelined emission (lookahead 2) ----
            # chunk 0 emitted up front; chunk ci+1's groups are spread across
            # ci's j-steps so PE fills the ACT-bound attention stretches.
            from collections import deque
            pipe = deque()       # pending (ci, pair, j, njt, et, yt)

            extra = []           # spreadable out-projection thunks

            def flush_one():
                if not pipe:
                    return
                pci, ppair, pj, pnjt, pet, pyt = pipe.popleft()
                emit_av(pci, ppair, pj, pnjt, pet, pyt)
                if pj == pnjt - 1:
                    emit_divides(pci, ppair, pyt)
                    if ppair == NPAIR - 1:
                        extra.extend(outproj_thunks(pci))

            # chunk 0: run only what round (0, pair 0) needs up front
            # (k/q quad 0 and the V tiles); spread the other quads into
            # ci=0's j-steps so ScalarE starts exps ~10us earlier.
            ch0 = chunk_thunks(0)
            for idx in (0, 4, 8, 9):
                ch0[idx]()
            # v2/v3 join the spread queue (needed only by AV j=2,3, several
            # steps in); k/q quads 1..3 follow in pair order
            rest0 = [ch0[i] for i in (10, 11, 1, 5, 2, 6, 3, 7)]
            pending = []
            for ci in range(NCH):
                if ci + 1 < NCH:
                    pending = (rest0 if ci == 0 else []) + chunk_thunks(ci + 1)
                n_steps = 16 * (ci + 1)
                n_thunks0 = len(pending)
                popped = 0
                step = 0
                for pair in range(NPAIR):
                    njt = 4 * (ci + 1)
                    yt = ps.tile([P, 2, QC], F32, tag="yt", bufs=1,
                                 name="yt")
                    for j in range(njt):
                        et = emit_qk(ci, pair, j, yt)
                        pipe.append((ci, pair, j, njt, et, yt))
                        if len(pipe) > 3:
                            flush_one()
                        step += 1
                        took = False
                        while (pending and
                               step * n_thunks0 >= (popped + 1) * n_steps):
                            pending.pop(0)()
                            popped += 1
                            took = True
                        if extra and (not took or len(extra) > 2):
                            extra.pop(0)()
                while pending:
                    pending.pop(0)()
            while pipe:
                flush_one()
            while extra:
                extra.pop(0)()

    nc.compile()
    return nc


def _prepare_inputs(x, w_attn, w_proj):
    # xT_b[ch, p, kc, t] = x[b, ch*PC + t, kc*128 + p]
    xT_all = [
        np.ascontiguousarray(
            x[b].reshape(NCH, PC, KC, P).transpose(0, 3, 2, 1)
        ).astype(np.float16)
        for b in range(B)
    ]

    kk = np.arange(P)[:, None]
    qq = np.arange(P)[None, :]
    tri = (qq >= kk).astype(np.float16)
    tri2 = np.ascontiguousarray(np.broadcast_to(tri[:, None, :], (P, 2, P)))
    ident = np.eye(P, dtype=np.float16)

    def wt(w):
        return np.ascontiguousarray(
            w.reshape(KC, P, D2).transpose(1, 0, 2)).astype(np.float16)

    in_maps = []
    for c in range(NCORES):
        b, hh = c // 2, c % 2
        cols = slice(hh * D2, (hh + 1) * D2)
        wqa = wt(w_attn[:, 0 * C:1 * C][:, cols])
        wka = wt(w_attn[:, 1 * C:2 * C][:, cols])
        wva = wt(w_attn[:, 2 * C:3 * C][:, cols])
        wpa = np.ascontiguousarray(
            w_proj[hh * D2:(hh + 1) * D2, :].reshape(NQ, P, C)
            .transpose(1, 0, 2)).astype(np.float16)
        in_maps.append({
            "xT": xT_all[b],
            "wq": wqa, "wk": wka, "wv": wva,
            "wp": wpa,
            "tri2": tri2,
        })
    return in_maps


def combine_outputs(outs):
    """outs: list of 8 per-core 'out' arrays [P, 4, 4, C] -> full [B, T, C]."""
    full = np.empty((B, T, C), dtype=np.float32)
    for b in range(B):
        acc = outs[2 * b].astype(np.float32) + outs[2 * b + 1].astype(np.float32)
        full[b] = acc.transpose(1, 2, 0, 3).reshape(T, C)
    return full


def kernel(x, w_attn, w_proj):
    from concourse.bass_utils import run_bass_kernel_spmd

    x = np.asarray(x, dtype=np.float32)
    w_attn = np.asarray(w_attn, dtype=np.float32)
    w_proj = np.asarray(w_proj, dtype=np.float32)

    if "nc" not in _CACHE:
        _CACHE["nc"] = build_program()
    nc = _CACHE["nc"]

    in_maps = _prepare_inputs(x, w_attn, w_proj)
    res = run_bass_kernel_spmd(nc, in_maps, list(range(NCORES)))
    return combine_outputs([r["out"] for r in res.results])
